# revision 8
# baseline (speedup 1.0000x reference)
"""Trainium2 Bass kernel for nn_EntEncoderFast (group-causal masked conv stack + GMM CDF table).

Strategy
--------
Key observation: the reference replicates the single image 3x along batch and runs
identical weights on every replica, so all three batch replicas are identical;
softmax over identical logits = 1/3 each and sum_m wts_m*cdf_m == cdf.  The whole
network therefore reduces to batch=1, and only the mu (p=1) and sigma (p=2) output
channels of the final conv are needed (128 of 192 channels).

Sharding: 8-way output-column strips (8 cols each) with *shrinking-window*
recompute — each core locally computes everything its strip needs (the window
shrinks by 2 cols/side per conv layer, 56 -> 8 over 12 layers), so there is NO
inter-core communication.  All cores run the identical SPMD program; per-core
differences are baked into the input DATA (x0 window slice, output strip).

Compute: 25-tap decomposition of the 5x5 convs; fp32r matmuls (full PE rate at
N>=256) accumulating in PSUM over taps x cin-chunks; masked weights prepared
host-side (group-causal zeros baked in); fully-zero (cout-chunk, cin-chunk, tap)
blocks skipped.  GMM tail (stable softplus via Exp/Ln, erf) on scalar-engine LUTs.
"""
import sys
for _p in ("/opt/trn_rl_repo", "/opt/pypackages"):
    if _p not in sys.path:
        sys.path.insert(0, _p)

import numpy as np

G = 64
CPN = 4
H, W = 32, 64
K = 5
BIN = 8
BIAS = 3.5
NMIX = 3
SCALE = 65536.0

NCORES = 8
STRIP = W // NCORES          # 8 output cols per core
NL = 12                      # total conv layers (1 in + 10 hid + 1 out)
BR, BC = H + 4, 60           # buffer rows (2+32+2), cols (2+56+2)

_CACHE = {}


def _group_mask(cout, cin, strict):
    g = np.arange(G)
    dg = g[None, :] - g[:, None]
    off = np.arange(K) - K // 2
    dd = off[:, None] + off[None, :]
    tot = dg[:, :, None, None] + dd[None, None, :, :]
    m = (tot < 0) if strict else (tot <= 0)
    m = np.repeat(np.repeat(m, cout, axis=0), cin, axis=1)
    return m.astype(np.float32)


TAPS = [(ki, kj, ki - 2, kj - 2) for ki in range(K) for kj in range(K)]


def _wl(layer):  # valid width of x_layer, layer 1..12
    return 8 + 4 * (NL - layer)


def _build(reps=1, prec="r"):
    import concourse.bacc as bacc
    import concourse.mybir as mybir
    import concourse.tile as tile

    F32 = mybir.dt.float32
    F32R = mybir.dt.float32r if prec == "r" else mybir.dt.float32
    AF = mybir.ActivationFunctionType
    AL = mybir.AluOpType

    nc = bacc.Bacc("TRN2", target_bir_lowering=False, debug=False,
                   num_devices=NCORES)

    x0_d = nc.dram_tensor("x0", [64, BR * BC], F32R, kind="ExternalInput").ap()
    win_d = nc.dram_tensor("win", [64, 25 * 2 * 128], F32R, kind="ExternalInput").ap()
    whid_d = nc.dram_tensor("whid", [10, 2, 128, 25 * 2 * 128], F32R, kind="ExternalInput").ap()
    wout_d = nc.dram_tensor("wout", [128, 25 * 2 * 128], F32R, kind="ExternalInput").ap()
    bin_d = nc.dram_tensor("bin", [128, 2], F32, kind="ExternalInput").ap()
    bhid_d = nc.dram_tensor("bhid", [128, 20], F32, kind="ExternalInput").ap()
    bout_d = nc.dram_tensor("bout", [128, 1], F32, kind="ExternalInput").ap()
    msk_d = nc.dram_tensor("msk", [128, BR * BC], F32R, kind="ExternalInput").ap()
    out_d = nc.dram_tensor("out", [64, H * STRIP * BIN], F32, kind="ExternalOutput").ap()

    with tile.TileContext(nc) as tc:
        with tc.tile_pool(name="xp", bufs=1) as xp, \
             tc.tile_pool(name="x0p", bufs=1) as x0p, \
             tc.tile_pool(name="wp", bufs=3) as wp, \
             tc.tile_pool(name="bp", bufs=1) as bp, \
             tc.tile_pool(name="ev", bufs=3) as evp, \
             tc.tile_pool(name="gm", bufs=1) as gm, \
             tc.tile_pool(name="ps", bufs=2, space="PSUM") as ps:

            x0_t = x0p.tile([64, BR * BC], F32R)
            msk_t = x0p.tile([128, BR * BC], F32R)
            nc.sync.dma_start(msk_t[:], msk_d)
            mskv = msk_t[:].rearrange("p (r c) -> p r c", c=BC)
            bin_t = bp.tile([128, 2], F32)
            bhid_t = bp.tile([128, 20], F32)
            bout_t = bp.tile([128, 1], F32)

            nc.sync.dma_start(x0_t[:], x0_d)
            nc.sync.dma_start(bin_t[:], bin_d)
            nc.sync.dma_start(bhid_t[:], bhid_d)
            nc.sync.dma_start(bout_t[:], bout_d)

            xb = []
            for i in range(3):
                xbt = xp.tile([128, 2 * BR * BC], F32R, tag=f"xb{i}")
                xb.append(xbt)
            for b in xb:
                nc.gpsimd.memset(b[:].bitcast(F32), 0.0)

            def xview(t):
                return t[:].rearrange("p (ch r c) -> p ch r c", ch=2, c=BC)

            for _rep in range(reps):
                # ---- layer 1: w_in (cin=64, K=64) -> x1 in xb[0] ----
                l = 1
                w_l = _wl(l)              # 52
                b0 = 2 + 2 * l
                x0v = x0_t[:].rearrange("p (r c) -> p r c", c=BC)
                win_t = wp.tile([128, 25 * 2 * 128], F32R, tag="whid")
                nc.sync.dma_start(win_t[:64, :], win_d)
                for o in range(2):
                    for r0 in range(0, H, 8):
                        nr = 8
                        pt = ps.tile([128, nr * w_l], F32, tag=f"ps{(r0 // 8) % 2}")
                        for ti, (ki, kj, di, dj) in enumerate(TAPS):
                            rhs = x0v[:, r0 + 2 + di: r0 + 2 + di + nr,
                                      b0 + dj: b0 + dj + w_l]
                            nc.tensor.matmul(
                                pt[:], win_t[:64, (ti * 2 + o) * 128:(ti * 2 + o + 1) * 128],
                                rhs, start=(ti == 0), stop=(ti == len(TAPS) - 1))
                        dst = xview(xb[0])[:, o, r0 + 2: r0 + 2 + nr, b0: b0 + w_l]
                        tmp = evp.tile([128, nr * w_l], F32, tag="evt")
                        nc.scalar.activation(tmp[:], pt[:], AF.Relu, bias=bin_t[:, o:o + 1])
                        nc.vector.tensor_mul(
                            dst, tmp[:].rearrange("p (r c) -> p r c", c=w_l),
                            mskv[:, r0 + 2: r0 + 2 + nr, b0: b0 + w_l])

                # ---- layers 2..11: hid convs ----
                for l in range(2, 12):
                    hw_l = l - 2
                    w_l = _wl(l)
                    b0 = 2 + 2 * l
                    src = xb[(l - 2) % 3]
                    dst_b = xb[(l - 1) % 3]
                    res_b = xb[(l - 3) % 3] if (l % 2 == 1) else None
                    xv = xview(src)
                    if w_l >= 32:
                        chunks = [(r, 8) for r in range(0, H, 8)]
                    elif w_l >= 16:
                        chunks = [(0, 16), (16, 16)]
                    else:
                        chunks = [(0, 32)]
                    for o in range(2):
                        wt = wp.tile([128, 25 * 2 * 128], F32R, tag="whid")
                        nc.sync.dma_start(wt[:], whid_d[hw_l, o])
                        mms = [(ti, ci) for ti in range(25) for ci in range(2)
                               if not (o == 0 and ci == 1 and TAPS[ti][2] + TAPS[ti][3] >= 0)]
                        for kidx, (r0, nr) in enumerate(chunks):
                            pt = ps.tile([128, nr * w_l], F32, tag=f"ps{kidx % 2}")
                            for mi, (ti, ci) in enumerate(mms):
                                di, dj = TAPS[ti][2], TAPS[ti][3]
                                rhs = xv[:, ci, r0 + 2 + di: r0 + 2 + di + nr,
                                         b0 + dj: b0 + dj + w_l]
                                nc.tensor.matmul(
                                    pt[:], wt[:, (ti * 2 + ci) * 128:(ti * 2 + ci + 1) * 128],
                                    rhs, start=(mi == 0), stop=(mi == len(mms) - 1))
                            ptv = pt[:].rearrange("p (r c) -> p r c", c=w_l)
                            dstv = xview(dst_b)[:, o, r0 + 2: r0 + 2 + nr, b0: b0 + w_l]
                            bias_ap = bhid_t[:, hw_l * 2 + o: hw_l * 2 + o + 1]
                            mv = mskv[:, r0 + 2: r0 + 2 + nr, b0: b0 + w_l]
                            tmp = evp.tile([128, nr * w_l], F32, tag="evt")
                            nc.scalar.activation(tmp[:], pt[:], AF.Relu, bias=bias_ap)
                            tmpv = tmp[:].rearrange("p (r c) -> p r c", c=w_l)
                            if res_b is None:
                                nc.vector.tensor_mul(dstv, tmpv, mv)
                            else:
                                tmp2 = evp.tile([128, nr * w_l], F32, tag="evt2")
                                nc.vector.tensor_mul(tmp2[:], tmp[:],
                                                     mskv[:, r0 + 2: r0 + 2 + nr, b0: b0 + w_l])
                                resv = xview(res_b)[:, o, r0 + 2: r0 + 2 + nr, b0: b0 + w_l]
                                nc.vector.tensor_add(
                                    dstv, tmp2[:].rearrange("p (r c) -> p r c", c=w_l), resv)

                # ---- layer 12: w_out -> y = [mu | sig_raw], [128, 32*8] ----
                l = 12
                b0 = 2 + 2 * l            # 26
                xv = xview(xb[(l - 2) % 3])
                wout_t = wp.tile([128, 25 * 2 * 128], F32R, tag="whid")
                nc.sync.dma_start(wout_t[:], wout_d)
                y_t = gm.tile([128, 256], F32, tag="y")
                pt = ps.tile([128, 256], F32, tag="ps0")
                mi = 0
                for ti, (ki, kj, di, dj) in enumerate(TAPS):
                    for ci in range(2):
                        rhs = xv[:, ci, 2 + di: 2 + di + H, b0 + dj: b0 + dj + STRIP]
                        nc.tensor.matmul(
                            pt[:], wout_t[:, (ti * 2 + ci) * 128:(ti * 2 + ci + 1) * 128],
                            rhs, start=(mi == 0), stop=(mi == 49))
                        mi += 1
                nc.scalar.activation(y_t[:], pt[:], AF.Identity, bias=bout_t[:, :])

                # ---- GMM tail on [64, 256] tiles ----
                mu = y_t[0:64, :]
                s_t = gm.tile([64, 256], F32, tag="sraw")
                nc.sync.dma_start(s_t[:], y_t[64:128, :])   # realign sig to partitions 0..63
                ab = gm.tile([64, 256], F32, tag="ab")
                ex = gm.tile([64, 256], F32, tag="ex")
                ln = gm.tile([64, 256], F32, tag="ln")
                rl = gm.tile([64, 256], F32, tag="rl")
                sg = gm.tile([64, 256], F32, tag="sg")
                rc = gm.tile([64, 256], F32, tag="rc")
                nc.scalar.activation(ab[:], s_t[:], AF.Abs)
                nc.scalar.activation(ex[:], ab[:], AF.Exp, scale=-1.0)
                nc.scalar.activation(ln[:], ex[:], AF.Ln, bias=1.0)
                nc.scalar.activation(rl[:], s_t[:], AF.Relu)
                nc.vector.scalar_tensor_tensor(sg[:], rl[:], 1e-6, ln[:], AL.add, AL.add)
                nc.vector.reciprocal(rc[:], sg[:])
                big = gm.tile([64, 256 * BIN], F32, tag="big")
                inv_sqrt2 = float(1.0 / np.sqrt(2.0))
                for k in range(BIN):
                    e_k = float(k) + 0.5 - BIAS
                    tk = gm.tile([64, 256], F32, tag="tk")
                    zk = gm.tile([64, 256], F32, tag="zk")
                    ek = gm.tile([64, 256], F32, tag="ek")
                    nc.vector.tensor_scalar(tk[:], mu, -inv_sqrt2, e_k * inv_sqrt2,
                                            AL.mult, AL.add)
                    nc.vector.tensor_mul(zk[:], tk[:], rc[:])
                    nc.scalar.activation(ek[:], zk[:], AF.Erf)
                    dstk = big[:].rearrange("p (px k) -> p px k", k=BIN)[:, :, k]
                    nc.vector.tensor_scalar(dstk, ek[:], 32768.0, 32768.0, AL.mult, AL.add)
                nc.sync.dma_start(out_d, big[:])

    nc.compile()
    return nc


def _host_prep(data, mask, w_in, b_in, w_hid, b_hid, w_out, b_out):
    m_in = _group_mask(CPN, 1, True)
    m_hid = _group_mask(CPN, CPN, False)
    m_out = _group_mask(NMIX, CPN, False)

    wm_in = (w_in * m_in).astype(np.float32)          # (256, 64, 5, 5)
    wm_hid = (w_hid * m_hid).astype(np.float32)       # (10, 256, 256, 5, 5)
    wm_out = (w_out * m_out).astype(np.float32)       # (192, 256, 5, 5)

    idx = np.concatenate([np.arange(G) * 3 + 1, np.arange(G) * 3 + 2])
    wo = wm_out[idx]                                  # (128, 256, 5, 5): [mu | sig]
    bo = b_out[idx].astype(np.float32)

    A_in = np.ascontiguousarray(
        wm_in.reshape(2, 128, 64, K, K).transpose(2, 3, 4, 0, 1)).reshape(64, 25 * 2 * 128)
    A_hid = np.ascontiguousarray(
        wm_hid.reshape(10, 2, 128, 2, 128, K, K).transpose(0, 1, 4, 5, 6, 3, 2)
    ).reshape(10, 2, 128, 25 * 2 * 128)
    A_out = np.ascontiguousarray(
        wo.reshape(128, 2, 128, K, K).transpose(2, 3, 4, 1, 0)).reshape(128, 25 * 2 * 128)

    bi = np.ascontiguousarray(b_in.astype(np.float32).reshape(2, 128).T)        # [128, 2]
    bh = np.ascontiguousarray(
        b_hid.astype(np.float32).reshape(10, 2, 128).transpose(2, 0, 1)).reshape(128, 20)

    tdata = ((data - BIAS) * mask)[0].astype(np.float32)   # (64, 32, 64)

    msks = []
    for c in range(NCORES):
        lo = STRIP * c - 24
        colm = np.zeros(BC, np.float32)
        for b in range(BC):
            col = lo + (b - 2)
            if 0 <= col < W:
                colm[b] = 1.0
        m = np.broadcast_to(colm, (128, BR, BC)).reshape(128, BR * BC)
        msks.append(np.ascontiguousarray(m))

    x0s = []
    for c in range(NCORES):
        buf = np.zeros((64, BR, BC), np.float32)
        lo = STRIP * c - 24
        s0, s1 = max(0, lo), min(W, lo + 56)
        if s1 > s0:
            buf[:, 2:2 + H, 2 + (s0 - lo): 2 + (s1 - lo)] = tdata[:, :, s0:s1]
        x0s.append(buf.reshape(64, BR * BC))
    return A_in, A_hid, A_out, bi, bh, bo.reshape(128, 1), x0s, msks


def kernel(data, mask, w_in, b_in, w_hid, b_hid, w_out, b_out, _reps=1, _prec="r"):
    from concourse.bass_utils import run_bass_kernel_spmd

    key = ("nc", _reps, _prec)
    if key not in _CACHE:
        _CACHE[key] = _build(_reps, _prec)
    nc = _CACHE[key]

    A_in, A_hid, A_out, bi, bh, bo, x0s, msks = _host_prep(
        data, mask, w_in, b_in, w_hid, b_hid, w_out, b_out)

    in_maps = [dict(x0=x0s[c], win=A_in, whid=A_hid, wout=A_out,
                    bin=bi, bhid=bh, bout=bo, msk=msks[c]) for c in range(NCORES)]
    import time as _time
    res = None
    for _try in range(4):
        try:
            res = run_bass_kernel_spmd(nc, in_maps, core_ids=list(range(NCORES)))
            break
        except Exception:
            # a wedged exec unit resets on the failed load; retry after a pause
            if _try == 3:
                raise
            _time.sleep(5)

    out = np.zeros((G, H, W, BIN), np.float32)
    for c in range(NCORES):
        out[:, :, STRIP * c:STRIP * (c + 1), :] = \
            res.results[c]["out"].reshape(G, H, STRIP, BIN)
    return out


if __name__ == "__main__":
    inp = dict(np.load("/root/problem/ref_inp.npz"))
    ref = np.load("/root/problem/ref_out.npy")
    out = kernel(**inp)
    d = out - ref
    print("rel (norm):", np.linalg.norm(d) / np.linalg.norm(ref))
    print("absmax:", np.abs(d).max())
    bad = np.abs(d) > 64
    print("frac |err|>64:", bad.mean())


# revision 9
# speedup vs baseline: 3.4848x; 3.4848x over previous
"""Trainium2 Bass kernel for nn_EntEncoderFast (group-causal masked conv stack + GMM CDF table).

Strategy
--------
Key observation: the reference replicates the single image 3x along batch and runs
identical weights on every replica, so all three batch replicas are identical;
softmax over identical logits = 1/3 each and sum_m wts_m*cdf_m == cdf.  The whole
network therefore reduces to batch=1, and only the mu (p=1) and sigma (p=2) output
channels of the final conv are needed (128 of 192 channels).

Sharding: 8-way output-column strips (8 cols each) with *shrinking-window*
recompute — each core locally computes everything its strip needs (the window
shrinks by 2 cols/side per conv layer, 56 -> 8 over 12 layers), so there is NO
inter-core communication.  All cores run the identical SPMD program; per-core
differences are baked into the input DATA (x0 window slice, output strip).

Compute: 25-tap decomposition of the 5x5 convs; fp32r matmuls (full PE rate at
N>=256) accumulating in PSUM over taps x cin-chunks; masked weights prepared
host-side (group-causal zeros baked in); fully-zero (cout-chunk, cin-chunk, tap)
blocks skipped.  GMM tail (stable softplus via Exp/Ln, erf) on scalar-engine LUTs.
"""
import sys
for _p in ("/opt/trn_rl_repo", "/opt/pypackages"):
    if _p not in sys.path:
        sys.path.insert(0, _p)

import numpy as np

G = 64
CPN = 4
H, W = 32, 64
K = 5
BIN = 8
BIAS = 3.5
NMIX = 3
SCALE = 65536.0

NCORES = 8
STRIP = W // NCORES          # 8 output cols per core
NL = 12                      # total conv layers (1 in + 10 hid + 1 out)
BR, BC = H + 4, 60           # buffer rows (2+32+2), cols (2+56+2)

_CACHE = {}


def _group_mask(cout, cin, strict):
    g = np.arange(G)
    dg = g[None, :] - g[:, None]
    off = np.arange(K) - K // 2
    dd = off[:, None] + off[None, :]
    tot = dg[:, :, None, None] + dd[None, None, :, :]
    m = (tot < 0) if strict else (tot <= 0)
    m = np.repeat(np.repeat(m, cout, axis=0), cin, axis=1)
    return m.astype(np.float32)


TAPS = [(ki, kj, ki - 2, kj - 2) for ki in range(K) for kj in range(K)]


def _wl(layer):  # valid width of x_layer, layer 1..12
    return 8 + 4 * (NL - layer)


def _build(reps=1, prec="r"):
    import concourse.bacc as bacc
    import concourse.mybir as mybir
    import concourse.tile as tile

    F32 = mybir.dt.float32
    F32R = mybir.dt.float32r if prec == "r" else mybir.dt.float32
    AF = mybir.ActivationFunctionType
    AL = mybir.AluOpType

    nc = bacc.Bacc("TRN2", target_bir_lowering=False, debug=False,
                   num_devices=NCORES)

    x0_d = nc.dram_tensor("x0", [64, BR * BC], F32R, kind="ExternalInput").ap()
    win_d = nc.dram_tensor("win", [64, 25 * 2 * 128], F32R, kind="ExternalInput").ap()
    whid_d = nc.dram_tensor("whid", [10, 2, 128, 25 * 2 * 128], F32R, kind="ExternalInput").ap()
    wout_d = nc.dram_tensor("wout", [128, 25 * 2 * 128], F32R, kind="ExternalInput").ap()
    bin_d = nc.dram_tensor("bin", [128, 2], F32, kind="ExternalInput").ap()
    bhid_d = nc.dram_tensor("bhid", [128, 20], F32, kind="ExternalInput").ap()
    bout_d = nc.dram_tensor("bout", [128, 1], F32, kind="ExternalInput").ap()
    msk_d = nc.dram_tensor("msk", [128, BR * BC], F32R, kind="ExternalInput").ap()
    out_d = nc.dram_tensor("out", [64, H * STRIP * BIN], F32, kind="ExternalOutput").ap()

    with tile.TileContext(nc) as tc:
        with tc.tile_pool(name="xp", bufs=1) as xp, \
             tc.tile_pool(name="x0p", bufs=1) as x0p, \
             tc.tile_pool(name="wp", bufs=3) as wp, \
             tc.tile_pool(name="bp", bufs=1) as bp, \
             tc.tile_pool(name="ev", bufs=3) as evp, \
             tc.tile_pool(name="gm", bufs=1) as gm, \
             tc.tile_pool(name="ps", bufs=2, space="PSUM") as ps:

            x0_t = x0p.tile([64, BR * BC], F32R)
            msk_t = x0p.tile([128, BR * BC], F32R)
            nc.sync.dma_start(msk_t[:], msk_d)
            mskv = msk_t[:].rearrange("p (r c) -> p r c", c=BC)
            bin_t = bp.tile([128, 2], F32)
            bhid_t = bp.tile([128, 20], F32)
            bout_t = bp.tile([128, 1], F32)

            nc.sync.dma_start(x0_t[:], x0_d)
            nc.sync.dma_start(bin_t[:], bin_d)
            nc.sync.dma_start(bhid_t[:], bhid_d)
            nc.sync.dma_start(bout_t[:], bout_d)

            xb = []
            for i in range(3):
                xbt = xp.tile([128, 2 * BR * BC], F32R, tag=f"xb{i}")
                xb.append(xbt)
            for b in xb:
                nc.gpsimd.memset(b[:].bitcast(F32), 0.0)

            def xview(t):
                return t[:].rearrange("p (ch r c) -> p ch r c", ch=2, c=BC)

            for _rep in range(reps):
                # ---- layer 1: w_in (cin=64, K=64) -> x1 in xb[0] ----
                l = 1
                w_l = _wl(l)              # 52
                b0 = 2 + 2 * l
                x0v = x0_t[:].rearrange("p (r c) -> p r c", c=BC)
                win_t = wp.tile([128, 25 * 2 * 128], F32R, tag="whid")
                nc.sync.dma_start(win_t[:64, :], win_d)
                for o in range(2):
                    for r0 in range(0, H, 8):
                        nr = 8
                        pt = ps.tile([128, nr * w_l], F32, tag=f"ps{(r0 // 8) % 2}")
                        for ti, (ki, kj, di, dj) in enumerate(TAPS):
                            rhs = x0v[:, r0 + 2 + di: r0 + 2 + di + nr,
                                      b0 + dj: b0 + dj + w_l]
                            nc.tensor.matmul(
                                pt[:], win_t[:64, (ti * 2 + o) * 128:(ti * 2 + o + 1) * 128],
                                rhs, start=(ti == 0), stop=(ti == len(TAPS) - 1))
                        dst = xview(xb[0])[:, o, r0 + 2: r0 + 2 + nr, b0: b0 + w_l]
                        tmp = evp.tile([128, nr * w_l], F32, tag="evt")
                        nc.scalar.activation(tmp[:], pt[:], AF.Relu, bias=bin_t[:, o:o + 1])
                        nc.vector.tensor_mul(
                            dst, tmp[:].rearrange("p (r c) -> p r c", c=w_l),
                            mskv[:, r0 + 2: r0 + 2 + nr, b0: b0 + w_l])

                # ---- layers 2..11: hid convs ----
                for l in range(2, 12):
                    hw_l = l - 2
                    w_l = _wl(l)
                    b0 = 2 + 2 * l
                    src = xb[(l - 2) % 3]
                    dst_b = xb[(l - 1) % 3]
                    res_b = xb[(l - 3) % 3] if (l % 2 == 1) else None
                    xv = xview(src)
                    if w_l >= 32:
                        chunks = [(r, 8) for r in range(0, H, 8)]
                    elif w_l >= 16:
                        chunks = [(0, 16), (16, 16)]
                    else:
                        chunks = [(0, 32)]
                    for o in range(2):
                        wt = wp.tile([128, 25 * 2 * 128], F32R, tag="whid")
                        nc.sync.dma_start(wt[:], whid_d[hw_l, o])
                        mms = [(ti, ci) for ti in range(25) for ci in range(2)
                               if not (o == 0 and ci == 1 and TAPS[ti][2] + TAPS[ti][3] >= 0)]
                        for kidx, (r0, nr) in enumerate(chunks):
                            pt = ps.tile([128, nr * w_l], F32, tag=f"ps{kidx % 2}")
                            for mi, (ti, ci) in enumerate(mms):
                                di, dj = TAPS[ti][2], TAPS[ti][3]
                                rhs = xv[:, ci, r0 + 2 + di: r0 + 2 + di + nr,
                                         b0 + dj: b0 + dj + w_l]
                                nc.tensor.matmul(
                                    pt[:], wt[:, (ti * 2 + ci) * 128:(ti * 2 + ci + 1) * 128],
                                    rhs, start=(mi == 0), stop=(mi == len(mms) - 1))
                            ptv = pt[:].rearrange("p (r c) -> p r c", c=w_l)
                            dstv = xview(dst_b)[:, o, r0 + 2: r0 + 2 + nr, b0: b0 + w_l]
                            bias_ap = bhid_t[:, hw_l * 2 + o: hw_l * 2 + o + 1]
                            mv = mskv[:, r0 + 2: r0 + 2 + nr, b0: b0 + w_l]
                            tmp = evp.tile([128, nr * w_l], F32, tag="evt")
                            nc.scalar.activation(tmp[:], pt[:], AF.Relu, bias=bias_ap)
                            tmpv = tmp[:].rearrange("p (r c) -> p r c", c=w_l)
                            if res_b is None:
                                nc.vector.tensor_mul(dstv, tmpv, mv)
                            else:
                                tmp2 = evp.tile([128, nr * w_l], F32, tag="evt2")
                                nc.vector.tensor_mul(tmp2[:], tmp[:],
                                                     mskv[:, r0 + 2: r0 + 2 + nr, b0: b0 + w_l])
                                resv = xview(res_b)[:, o, r0 + 2: r0 + 2 + nr, b0: b0 + w_l]
                                nc.vector.tensor_add(
                                    dstv, tmp2[:].rearrange("p (r c) -> p r c", c=w_l), resv)

                # ---- layer 12: w_out -> y = [mu | sig_raw], [128, 32*8] ----
                l = 12
                b0 = 2 + 2 * l            # 26
                xv = xview(xb[(l - 2) % 3])
                wout_t = wp.tile([128, 25 * 2 * 128], F32R, tag="whid")
                nc.sync.dma_start(wout_t[:], wout_d)
                y_t = gm.tile([128, 256], F32, tag="y")
                pt = ps.tile([128, 256], F32, tag="ps0")
                mi = 0
                for ti, (ki, kj, di, dj) in enumerate(TAPS):
                    for ci in range(2):
                        rhs = xv[:, ci, 2 + di: 2 + di + H, b0 + dj: b0 + dj + STRIP]
                        nc.tensor.matmul(
                            pt[:], wout_t[:, (ti * 2 + ci) * 128:(ti * 2 + ci + 1) * 128],
                            rhs, start=(mi == 0), stop=(mi == 49))
                        mi += 1
                nc.scalar.activation(y_t[:], pt[:], AF.Identity, bias=bout_t[:, :])

                # ---- GMM tail on [64, 256] tiles ----
                mu = y_t[0:64, :]
                s_t = gm.tile([64, 256], F32, tag="sraw")
                nc.sync.dma_start(s_t[:], y_t[64:128, :])   # realign sig to partitions 0..63
                ab = gm.tile([64, 256], F32, tag="ab")
                ex = gm.tile([64, 256], F32, tag="ex")
                ln = gm.tile([64, 256], F32, tag="ln")
                rl = gm.tile([64, 256], F32, tag="rl")
                sg = gm.tile([64, 256], F32, tag="sg")
                rc = gm.tile([64, 256], F32, tag="rc")
                nc.scalar.activation(ab[:], s_t[:], AF.Abs)
                nc.scalar.activation(ex[:], ab[:], AF.Exp, scale=-1.0)
                nc.scalar.activation(ln[:], ex[:], AF.Ln, bias=1.0)
                nc.scalar.activation(rl[:], s_t[:], AF.Relu)
                nc.vector.scalar_tensor_tensor(sg[:], rl[:], 1e-6, ln[:], AL.add, AL.add)
                nc.vector.reciprocal(rc[:], sg[:])
                big = gm.tile([64, 256 * BIN], F32, tag="big")
                inv_sqrt2 = float(1.0 / np.sqrt(2.0))
                for k in range(BIN):
                    e_k = float(k) + 0.5 - BIAS
                    tk = gm.tile([64, 256], F32, tag="tk")
                    zk = gm.tile([64, 256], F32, tag="zk")
                    ek = gm.tile([64, 256], F32, tag="ek")
                    nc.vector.tensor_scalar(tk[:], mu, -inv_sqrt2, e_k * inv_sqrt2,
                                            AL.mult, AL.add)
                    nc.vector.tensor_mul(zk[:], tk[:], rc[:])
                    nc.scalar.activation(ek[:], zk[:], AF.Erf)
                    dstk = big[:].rearrange("p (px k) -> p px k", k=BIN)[:, :, k]
                    nc.vector.tensor_scalar(dstk, ek[:], 32768.0, 32768.0, AL.mult, AL.add)
                nc.sync.dma_start(out_d, big[:])

    nc.compile()
    return nc


def _host_prep(data, mask, w_in, b_in, w_hid, b_hid, w_out, b_out):
    data, mask, w_in, b_in, w_hid, b_hid, w_out, b_out = [
        np.asarray(a, dtype=np.float32)
        for a in (data, mask, w_in, b_in, w_hid, b_hid, w_out, b_out)]
    m_in = _group_mask(CPN, 1, True)
    m_hid = _group_mask(CPN, CPN, False)
    m_out = _group_mask(NMIX, CPN, False)

    wm_in = (w_in * m_in).astype(np.float32)          # (256, 64, 5, 5)
    wm_hid = (w_hid * m_hid).astype(np.float32)       # (10, 256, 256, 5, 5)
    wm_out = (w_out * m_out).astype(np.float32)       # (192, 256, 5, 5)

    idx = np.concatenate([np.arange(G) * 3 + 1, np.arange(G) * 3 + 2])
    wo = wm_out[idx]                                  # (128, 256, 5, 5): [mu | sig]
    bo = b_out[idx].astype(np.float32)

    A_in = np.ascontiguousarray(
        wm_in.reshape(2, 128, 64, K, K).transpose(2, 3, 4, 0, 1)).reshape(64, 25 * 2 * 128)
    A_hid = np.ascontiguousarray(
        wm_hid.reshape(10, 2, 128, 2, 128, K, K).transpose(0, 1, 4, 5, 6, 3, 2)
    ).reshape(10, 2, 128, 25 * 2 * 128)
    A_out = np.ascontiguousarray(
        wo.reshape(128, 2, 128, K, K).transpose(2, 3, 4, 1, 0)).reshape(128, 25 * 2 * 128)

    bi = np.ascontiguousarray(b_in.astype(np.float32).reshape(2, 128).T)        # [128, 2]
    bh = np.ascontiguousarray(
        b_hid.astype(np.float32).reshape(10, 2, 128).transpose(2, 0, 1)).reshape(128, 20)

    tdata = ((data - BIAS) * mask)[0].astype(np.float32)   # (64, 32, 64)

    msks = []
    for c in range(NCORES):
        lo = STRIP * c - 24
        colm = np.zeros(BC, np.float32)
        for b in range(BC):
            col = lo + (b - 2)
            if 0 <= col < W:
                colm[b] = 1.0
        m = np.broadcast_to(colm, (128, BR, BC)).reshape(128, BR * BC)
        msks.append(np.ascontiguousarray(m))

    x0s = []
    for c in range(NCORES):
        buf = np.zeros((64, BR, BC), np.float32)
        lo = STRIP * c - 24
        s0, s1 = max(0, lo), min(W, lo + 56)
        if s1 > s0:
            buf[:, 2:2 + H, 2 + (s0 - lo): 2 + (s1 - lo)] = tdata[:, :, s0:s1]
        x0s.append(buf.reshape(64, BR * BC))
    return A_in, A_hid, A_out, bi, bh, bo.reshape(128, 1), x0s, msks


def kernel(data, mask, w_in, b_in, w_hid, b_hid, w_out, b_out, _reps=1, _prec="r"):
    from concourse.bass_utils import run_bass_kernel_spmd

    key = ("nc", _reps, _prec)
    if key not in _CACHE:
        _CACHE[key] = _build(_reps, _prec)
    nc = _CACHE[key]

    A_in, A_hid, A_out, bi, bh, bo, x0s, msks = _host_prep(
        data, mask, w_in, b_in, w_hid, b_hid, w_out, b_out)

    in_maps = [dict(x0=x0s[c], win=A_in, whid=A_hid, wout=A_out,
                    bin=bi, bhid=bh, bout=bo, msk=msks[c]) for c in range(NCORES)]
    import time as _time
    res = None
    for _try in range(4):
        try:
            res = run_bass_kernel_spmd(nc, in_maps, core_ids=list(range(NCORES)))
            break
        except Exception:
            # a wedged exec unit resets on the failed load; retry after a pause
            if _try == 3:
                raise
            _time.sleep(5)

    out = np.zeros((G, H, W, BIN), np.float32)
    for c in range(NCORES):
        out[:, :, STRIP * c:STRIP * (c + 1), :] = \
            res.results[c]["out"].reshape(G, H, STRIP, BIN)
    return out


if __name__ == "__main__":
    inp = dict(np.load("/root/problem/ref_inp.npz"))
    ref = np.load("/root/problem/ref_out.npy")
    out = kernel(**inp)
    d = out - ref
    print("rel (norm):", np.linalg.norm(d) / np.linalg.norm(ref))
    print("absmax:", np.abs(d).max())
    bad = np.abs(d) > 64
    print("frac |err|>64:", bad.mean())


# revision 10
# speedup vs baseline: 4.6801x; 1.3430x over previous
"""Trainium2 Bass kernel for nn_EntEncoderFast (group-causal masked conv stack + GMM CDF table).

Strategy
--------
Key observation: the reference replicates the single image 3x along batch and runs
identical weights on every replica, so all three batch replicas are identical;
softmax over identical logits = 1/3 each and sum_m wts_m*cdf_m == cdf.  The whole
network therefore reduces to batch=1, and only the mu (p=1) and sigma (p=2) output
channels of the final conv are needed (128 of 192 channels).

Sharding: 8-way output-column strips (8 cols each) with *shrinking-window*
recompute — each core locally computes everything its strip needs (the window
shrinks by 2 cols/side per conv layer, 56 -> 8 over 12 layers), so there is NO
inter-core communication.  All cores run the identical SPMD program; per-core
differences are baked into the input DATA (x0 window slice, output strip).

Compute: 25-tap decomposition of the 5x5 convs; fp32r matmuls (full PE rate at
N>=256) accumulating in PSUM over taps x cin-chunks; masked weights prepared
host-side (group-causal zeros baked in); fully-zero (cout-chunk, cin-chunk, tap)
blocks skipped.  GMM tail (stable softplus via Exp/Ln, erf) on scalar-engine LUTs.
"""
import sys
for _p in ("/opt/trn_rl_repo", "/opt/pypackages"):
    if _p not in sys.path:
        sys.path.insert(0, _p)

import numpy as np

G = 64
CPN = 4
H, W = 32, 64
K = 5
BIN = 8
BIAS = 3.5
NMIX = 3
SCALE = 65536.0

NCORES = 8
STRIP = W // NCORES          # 8 output cols per core
NL = 12                      # total conv layers (1 in + 10 hid + 1 out)
BR, BC = H + 4, 60           # buffer rows (2+32+2), cols (2+56+2)

_CACHE = {}


def _group_mask(cout, cin, strict):
    g = np.arange(G)
    dg = g[None, :] - g[:, None]
    off = np.arange(K) - K // 2
    dd = off[:, None] + off[None, :]
    tot = dg[:, :, None, None] + dd[None, None, :, :]
    m = (tot < 0) if strict else (tot <= 0)
    m = np.repeat(np.repeat(m, cout, axis=0), cin, axis=1)
    return m.astype(np.float32)


TAPS = [(ki, kj, ki - 2, kj - 2) for ki in range(K) for kj in range(K)]


def _wl(layer):  # valid width of x_layer, layer 1..12
    return 8 + 4 * (NL - layer)


def _build(reps=1, prec="r"):
    import concourse.bacc as bacc
    import concourse.mybir as mybir
    import concourse.tile as tile

    F32 = mybir.dt.float32
    F32R = mybir.dt.float32r if prec == "r" else mybir.dt.float32
    AF = mybir.ActivationFunctionType
    AL = mybir.AluOpType

    nc = bacc.Bacc("TRN2", target_bir_lowering=False, debug=False,
                   num_devices=NCORES)

    x0_d = nc.dram_tensor("x0", [128, BR * BC], F32R, kind="ExternalInput").ap()
    win_d = nc.dram_tensor("win", [128, 5 * 3 * 2 * 128], F32R, kind="ExternalInput").ap()
    whid_d = nc.dram_tensor("whid", [10, 2, 128, 25 * 2 * 128], F32R, kind="ExternalInput").ap()
    wout_d = nc.dram_tensor("wout", [128, 25 * 2 * 128], F32R, kind="ExternalInput").ap()
    bin_d = nc.dram_tensor("bin", [128, 2], F32, kind="ExternalInput").ap()
    bhid_d = nc.dram_tensor("bhid", [128, 20], F32, kind="ExternalInput").ap()
    bout_d = nc.dram_tensor("bout", [128, 1], F32, kind="ExternalInput").ap()
    msk_d = nc.dram_tensor("msk", [128, BR * BC], F32R, kind="ExternalInput").ap()
    out_d = nc.dram_tensor("out", [64, H * STRIP * BIN], F32, kind="ExternalOutput").ap()

    with tile.TileContext(nc) as tc:
        with tc.tile_pool(name="xp", bufs=1) as xp, \
             tc.tile_pool(name="x0p", bufs=1) as x0p, \
             tc.tile_pool(name="wp", bufs=3) as wp, \
             tc.tile_pool(name="bp", bufs=1) as bp, \
             tc.tile_pool(name="ev", bufs=3) as evp, \
             tc.tile_pool(name="gm", bufs=1) as gm, \
             tc.tile_pool(name="ps", bufs=2, space="PSUM") as ps:

            x0_t = x0p.tile([128, BR * BC], F32R)
            msk_t = x0p.tile([128, BR * BC], F32R)
            nc.sync.dma_start(msk_t[:], msk_d)
            mskv = msk_t[:].rearrange("p (r c) -> p r c", c=BC)
            bin_t = bp.tile([128, 2], F32)
            bhid_t = bp.tile([128, 20], F32)
            bout_t = bp.tile([128, 1], F32)

            nc.sync.dma_start(x0_t[:], x0_d)
            nc.sync.dma_start(bin_t[:], bin_d)
            nc.sync.dma_start(bhid_t[:], bhid_d)
            nc.sync.dma_start(bout_t[:], bout_d)

            xb = []
            for i in range(3):
                xbt = xp.tile([128, 2 * BR * BC], F32R, tag=f"xb{i}")
                xb.append(xbt)
            for b in xb:
                nc.gpsimd.memset(b[:].bitcast(F32), 0.0)

            def xview(t):
                return t[:].rearrange("p (ch r c) -> p ch r c", ch=2, c=BC)

            for _rep in range(reps):
                # ---- layer 1: w_in (cin=64, K=64) -> x1 in xb[0] ----
                l = 1
                w_l = _wl(l)              # 52
                b0 = 2 + 2 * l
                x0v = x0_t[:].rearrange("p (r c) -> p r c", c=BC)
                win_t = wp.tile([128, 25 * 2 * 128], F32R, tag="whid")
                nc.sync.dma_start(win_t[:, :3840], win_d)
                # tap-pair packing: partitions 64-127 of x0 hold a 1-col-left-shifted
                # copy, so one K=128 matmul evaluates taps (ki,kj) and (ki,kj+1).
                for o in range(2):
                    for r0 in range(0, H, 8):
                        nr = 8
                        pt = ps.tile([128, nr * w_l], F32, tag=f"ps{(r0 // 8) % 2}")
                        mi = 0
                        for ki in range(K):
                            di = ki - 2
                            for si, dj in enumerate((-2, 0, 2)):
                                rhs = x0v[:, r0 + 2 + di: r0 + 2 + di + nr,
                                          b0 + dj: b0 + dj + w_l]
                                nc.tensor.matmul(
                                    pt[:],
                                    win_t[:, ((ki * 3 + si) * 2 + o) * 128:
                                          ((ki * 3 + si) * 2 + o + 1) * 128],
                                    rhs, start=(mi == 0), stop=(mi == 14))
                                mi += 1
                        dst = xview(xb[0])[:, o, r0 + 2: r0 + 2 + nr, b0: b0 + w_l]
                        tmp = evp.tile([128, nr * w_l], F32, tag="evt")
                        nc.scalar.activation(tmp[:], pt[:], AF.Relu, bias=bin_t[:, o:o + 1])
                        nc.vector.tensor_mul(
                            dst, tmp[:].rearrange("p (r c) -> p r c", c=w_l),
                            mskv[:, r0 + 2: r0 + 2 + nr, b0: b0 + w_l])

                # ---- layers 2..11: hid convs ----
                for l in range(2, 12):
                    hw_l = l - 2
                    w_l = _wl(l)
                    b0 = 2 + 2 * l
                    src = xb[(l - 2) % 3]
                    dst_b = xb[(l - 1) % 3]
                    res_b = xb[(l - 3) % 3] if (l % 2 == 1) else None
                    xv = xview(src)
                    if w_l >= 32:
                        chunks = [(r, 8) for r in range(0, H, 8)]
                    elif w_l >= 16:
                        chunks = [(0, 16), (16, 16)]
                    else:
                        chunks = [(0, 32)]
                    for o in range(2):
                        wt = wp.tile([128, 25 * 2 * 128], F32R, tag="whid")
                        nc.sync.dma_start(wt[:], whid_d[hw_l, o])
                        mms = [(ti, ci) for ti in range(25) for ci in range(2)
                               if not (o == 0 and ci == 1 and TAPS[ti][2] + TAPS[ti][3] >= 0)]
                        for kidx, (r0, nr) in enumerate(chunks):
                            pt = ps.tile([128, nr * w_l], F32, tag=f"ps{kidx % 2}")
                            for mi, (ti, ci) in enumerate(mms):
                                di, dj = TAPS[ti][2], TAPS[ti][3]
                                rhs = xv[:, ci, r0 + 2 + di: r0 + 2 + di + nr,
                                         b0 + dj: b0 + dj + w_l]
                                nc.tensor.matmul(
                                    pt[:], wt[:, (ti * 2 + ci) * 128:(ti * 2 + ci + 1) * 128],
                                    rhs, start=(mi == 0), stop=(mi == len(mms) - 1))
                            ptv = pt[:].rearrange("p (r c) -> p r c", c=w_l)
                            dstv = xview(dst_b)[:, o, r0 + 2: r0 + 2 + nr, b0: b0 + w_l]
                            bias_ap = bhid_t[:, hw_l * 2 + o: hw_l * 2 + o + 1]
                            mv = mskv[:, r0 + 2: r0 + 2 + nr, b0: b0 + w_l]
                            tmp = evp.tile([128, nr * w_l], F32, tag="evt")
                            nc.scalar.activation(tmp[:], pt[:], AF.Relu, bias=bias_ap)
                            tmpv = tmp[:].rearrange("p (r c) -> p r c", c=w_l)
                            if res_b is None:
                                nc.vector.tensor_mul(dstv, tmpv, mv)
                            else:
                                tmp2 = evp.tile([128, nr * w_l], F32, tag="evt2")
                                nc.vector.tensor_mul(tmp2[:], tmp[:],
                                                     mskv[:, r0 + 2: r0 + 2 + nr, b0: b0 + w_l])
                                resv = xview(res_b)[:, o, r0 + 2: r0 + 2 + nr, b0: b0 + w_l]
                                nc.vector.tensor_add(
                                    dstv, tmp2[:].rearrange("p (r c) -> p r c", c=w_l), resv)

                # ---- layer 12: w_out -> y = [mu | sig_raw], [128, 32*8] ----
                l = 12
                b0 = 2 + 2 * l            # 26
                xv = xview(xb[(l - 2) % 3])
                wout_t = wp.tile([128, 25 * 2 * 128], F32R, tag="whid")
                nc.sync.dma_start(wout_t[:], wout_d)
                y_t = gm.tile([128, 256], F32, tag="y")
                pt = ps.tile([128, 256], F32, tag="ps0")
                mi = 0
                for ti, (ki, kj, di, dj) in enumerate(TAPS):
                    for ci in range(2):
                        rhs = xv[:, ci, 2 + di: 2 + di + H, b0 + dj: b0 + dj + STRIP]
                        nc.tensor.matmul(
                            pt[:], wout_t[:, (ti * 2 + ci) * 128:(ti * 2 + ci + 1) * 128],
                            rhs, start=(mi == 0), stop=(mi == 49))
                        mi += 1
                nc.scalar.activation(y_t[:], pt[:], AF.Identity, bias=bout_t[:, :])

                # ---- GMM tail on [64, 256] tiles ----
                mu = y_t[0:64, :]
                s_t = gm.tile([64, 256], F32, tag="sraw")
                nc.sync.dma_start(s_t[:], y_t[64:128, :])   # realign sig to partitions 0..63
                ab = gm.tile([64, 256], F32, tag="ab")
                ex = gm.tile([64, 256], F32, tag="ex")
                ln = gm.tile([64, 256], F32, tag="ln")
                rl = gm.tile([64, 256], F32, tag="rl")
                sg = gm.tile([64, 256], F32, tag="sg")
                rc = gm.tile([64, 256], F32, tag="rc")
                nc.scalar.activation(ab[:], s_t[:], AF.Abs)
                nc.scalar.activation(ex[:], ab[:], AF.Exp, scale=-1.0)
                nc.scalar.activation(ln[:], ex[:], AF.Ln, bias=1.0)
                nc.scalar.activation(rl[:], s_t[:], AF.Relu)
                nc.vector.scalar_tensor_tensor(sg[:], rl[:], 1e-6, ln[:], AL.add, AL.add)
                nc.vector.reciprocal(rc[:], sg[:])
                big = gm.tile([64, 256 * BIN], F32, tag="big")
                inv_sqrt2 = float(1.0 / np.sqrt(2.0))
                for k in range(BIN):
                    e_k = float(k) + 0.5 - BIAS
                    tk = gm.tile([64, 256], F32, tag="tk")
                    zk = gm.tile([64, 256], F32, tag="zk")
                    ek = gm.tile([64, 256], F32, tag="ek")
                    nc.vector.tensor_scalar(tk[:], mu, -inv_sqrt2, e_k * inv_sqrt2,
                                            AL.mult, AL.add)
                    nc.vector.tensor_mul(zk[:], tk[:], rc[:])
                    nc.scalar.activation(ek[:], zk[:], AF.Erf)
                    dstk = big[:].rearrange("p (px k) -> p px k", k=BIN)[:, :, k]
                    nc.vector.tensor_scalar(dstk, ek[:], 32768.0, 32768.0, AL.mult, AL.add)
                nc.sync.dma_start(out_d, big[:])

    nc.compile()
    return nc


def _host_prep(data, mask, w_in, b_in, w_hid, b_hid, w_out, b_out):
    data, mask, w_in, b_in, w_hid, b_hid, w_out, b_out = [
        np.asarray(a, dtype=np.float32)
        for a in (data, mask, w_in, b_in, w_hid, b_hid, w_out, b_out)]
    m_in = _group_mask(CPN, 1, True)
    m_hid = _group_mask(CPN, CPN, False)
    m_out = _group_mask(NMIX, CPN, False)

    wm_in = (w_in * m_in).astype(np.float32)          # (256, 64, 5, 5)
    wm_hid = (w_hid * m_hid).astype(np.float32)       # (10, 256, 256, 5, 5)
    wm_out = (w_out * m_out).astype(np.float32)       # (192, 256, 5, 5)

    idx = np.concatenate([np.arange(G) * 3 + 1, np.arange(G) * 3 + 2])
    wo = wm_out[idx]                                  # (128, 256, 5, 5): [mu | sig]
    bo = b_out[idx].astype(np.float32)

    # pair-packed w_in: [p(128), (ki, s, o, col)]; s=0:(kj 0,1), s=1:(kj 2,3), s=2:(kj 4,-)
    wiT = wm_in.reshape(2, 128, 64, K, K).transpose(2, 3, 4, 0, 1)  # (p64, ki, kj, o, col)
    A_in = np.zeros((128, K, 3, 2, 128), np.float32)
    for si, kjs in enumerate(((0, 1), (2, 3), (4,))):
        A_in[0:64, :, si] = wiT[:, :, kjs[0]]
        if len(kjs) > 1:
            A_in[64:128, :, si] = wiT[:, :, kjs[1]]
    A_in = np.ascontiguousarray(A_in).reshape(128, K * 3 * 2 * 128)
    A_hid = np.ascontiguousarray(
        wm_hid.reshape(10, 2, 128, 2, 128, K, K).transpose(0, 1, 4, 5, 6, 3, 2)
    ).reshape(10, 2, 128, 25 * 2 * 128)
    A_out = np.ascontiguousarray(
        wo.reshape(128, 2, 128, K, K).transpose(2, 3, 4, 1, 0)).reshape(128, 25 * 2 * 128)

    bi = np.ascontiguousarray(b_in.astype(np.float32).reshape(2, 128).T)        # [128, 2]
    bh = np.ascontiguousarray(
        b_hid.astype(np.float32).reshape(10, 2, 128).transpose(2, 0, 1)).reshape(128, 20)

    tdata = ((data - BIAS) * mask)[0].astype(np.float32)   # (64, 32, 64)

    msks = []
    for c in range(NCORES):
        lo = STRIP * c - 24
        colm = np.zeros(BC, np.float32)
        for b in range(BC):
            col = lo + (b - 2)
            if 0 <= col < W:
                colm[b] = 1.0
        m = np.broadcast_to(colm, (128, BR, BC)).reshape(128, BR * BC)
        msks.append(np.ascontiguousarray(m))

    x0s = []
    for c in range(NCORES):
        buf = np.zeros((128, BR, BC), np.float32)
        lo = STRIP * c - 24
        s0, s1 = max(0, lo), min(W, lo + 56)
        if s1 > s0:
            buf[0:64, 2:2 + H, 2 + (s0 - lo): 2 + (s1 - lo)] = tdata[:, :, s0:s1]
        buf[64:128, :, :-1] = buf[0:64, :, 1:]   # shifted copy for tap-pair packing
        x0s.append(buf.reshape(128, BR * BC))
    return A_in, A_hid, A_out, bi, bh, bo.reshape(128, 1), x0s, msks


def kernel(data, mask, w_in, b_in, w_hid, b_hid, w_out, b_out, _reps=1, _prec="r"):
    from concourse.bass_utils import run_bass_kernel_spmd

    key = ("nc", _reps, _prec)
    if key not in _CACHE:
        _CACHE[key] = _build(_reps, _prec)
    nc = _CACHE[key]

    A_in, A_hid, A_out, bi, bh, bo, x0s, msks = _host_prep(
        data, mask, w_in, b_in, w_hid, b_hid, w_out, b_out)

    in_maps = [dict(x0=x0s[c], win=A_in, whid=A_hid, wout=A_out,
                    bin=bi, bhid=bh, bout=bo, msk=msks[c]) for c in range(NCORES)]
    import time as _time
    res = None
    for _try in range(4):
        try:
            res = run_bass_kernel_spmd(nc, in_maps, core_ids=list(range(NCORES)))
            break
        except Exception:
            # a wedged exec unit resets on the failed load; retry after a pause
            if _try == 3:
                raise
            _time.sleep(5)

    out = np.zeros((G, H, W, BIN), np.float32)
    for c in range(NCORES):
        out[:, :, STRIP * c:STRIP * (c + 1), :] = \
            res.results[c]["out"].reshape(G, H, STRIP, BIN)
    return out


if __name__ == "__main__":
    inp = dict(np.load("/root/problem/ref_inp.npz"))
    ref = np.load("/root/problem/ref_out.npy")
    out = kernel(**inp)
    d = out - ref
    print("rel (norm):", np.linalg.norm(d) / np.linalg.norm(ref))
    print("absmax:", np.abs(d).max())
    bad = np.abs(d) > 64
    print("frac |err|>64:", bad.mean())


# revision 11
# speedup vs baseline: 5.1971x; 1.1105x over previous
"""Trainium2 Bass kernel for nn_EntEncoderFast (group-causal masked conv stack + GMM CDF table).

Strategy
--------
Key observation: the reference replicates the single image 3x along batch and runs
identical weights on every replica, so all three batch replicas are identical;
softmax over identical logits = 1/3 each and sum_m wts_m*cdf_m == cdf.  The whole
network therefore reduces to batch=1, and only the mu (p=1) and sigma (p=2) output
channels of the final conv are needed (128 of 192 channels).

Sharding: 8-way output-column strips (8 cols each) with *shrinking-window*
recompute — each core locally computes everything its strip needs (the window
shrinks by 2 cols/side per conv layer, 56 -> 8 over 12 layers), so there is NO
inter-core communication.  All cores run the identical SPMD program; per-core
differences are baked into the input DATA (x0 window slice, output strip).

Compute: 25-tap decomposition of the 5x5 convs; fp32r matmuls (full PE rate at
N>=256) accumulating in PSUM over taps x cin-chunks; masked weights prepared
host-side (group-causal zeros baked in); fully-zero (cout-chunk, cin-chunk, tap)
blocks skipped.  GMM tail (stable softplus via Exp/Ln, erf) on scalar-engine LUTs.
"""
import sys
for _p in ("/opt/trn_rl_repo", "/opt/pypackages"):
    if _p not in sys.path:
        sys.path.insert(0, _p)

import numpy as np

G = 64
CPN = 4
H, W = 32, 64
K = 5
BIN = 8
BIAS = 3.5
NMIX = 3
SCALE = 65536.0

NCORES = 8
STRIP = W // NCORES          # 8 output cols per core
NL = 12                      # total conv layers (1 in + 10 hid + 1 out)
BR, BC = H + 4, 60           # buffer rows (2+32+2), cols (2+56+2)

_CACHE = {}


def _group_mask(cout, cin, strict):
    g = np.arange(G)
    dg = g[None, :] - g[:, None]
    off = np.arange(K) - K // 2
    dd = off[:, None] + off[None, :]
    tot = dg[:, :, None, None] + dd[None, None, :, :]
    m = (tot < 0) if strict else (tot <= 0)
    m = np.repeat(np.repeat(m, cout, axis=0), cin, axis=1)
    return m.astype(np.float32)


TAPS = [(ki, kj, ki - 2, kj - 2) for ki in range(K) for kj in range(K)]


def _wl(layer):  # valid width of x_layer, layer 1..12
    return 8 + 4 * (NL - layer)


def _build(reps=1, prec="r"):
    import concourse.bacc as bacc
    import concourse.mybir as mybir
    import concourse.tile as tile

    F32 = mybir.dt.float32
    F32R = mybir.dt.float32r if prec == "r" else mybir.dt.float32
    AF = mybir.ActivationFunctionType
    AL = mybir.AluOpType

    nc = bacc.Bacc("TRN2", target_bir_lowering=False, debug=False,
                   num_devices=NCORES)

    x0_d = nc.dram_tensor("x0", [128, BR * BC], F32R, kind="ExternalInput").ap()
    win_d = nc.dram_tensor("win", [128, 5 * 3 * 2 * 128], F32R, kind="ExternalInput").ap()
    whid_d = nc.dram_tensor("whid", [10, 2, 128, 25 * 2 * 128], F32R, kind="ExternalInput").ap()
    wout_d = nc.dram_tensor("wout", [128, 25 * 2 * 128], F32R, kind="ExternalInput").ap()
    bin_d = nc.dram_tensor("bin", [128, 2], F32, kind="ExternalInput").ap()
    bhid_d = nc.dram_tensor("bhid", [128, 20], F32, kind="ExternalInput").ap()
    bout_d = nc.dram_tensor("bout", [128, 1], F32, kind="ExternalInput").ap()
    msk_d = nc.dram_tensor("msk", [128, BR * BC], F32R, kind="ExternalInput").ap()
    out_d = nc.dram_tensor("out", [64, H * STRIP * BIN], F32, kind="ExternalOutput").ap()

    with tile.TileContext(nc) as tc:
        with tc.tile_pool(name="xp", bufs=1) as xp, \
             tc.tile_pool(name="x0p", bufs=1) as x0p, \
             tc.tile_pool(name="wp", bufs=3) as wp, \
             tc.tile_pool(name="bp", bufs=1) as bp, \
             tc.tile_pool(name="ev", bufs=3) as evp, \
             tc.tile_pool(name="gm", bufs=1) as gm, \
             tc.tile_pool(name="ps", bufs=2, space="PSUM") as ps:

            x0_t = x0p.tile([128, BR * BC], F32R)
            msk_t = x0p.tile([128, BR * BC], F32R)
            nc.sync.dma_start(msk_t[:], msk_d)
            mskv = msk_t[:].rearrange("p (r c) -> p r c", c=BC)
            bin_t = bp.tile([128, 2], F32)
            bhid_t = bp.tile([128, 20], F32)
            bout_t = bp.tile([128, 1], F32)

            nc.sync.dma_start(x0_t[:], x0_d)
            nc.sync.dma_start(bin_t[:], bin_d)
            nc.sync.dma_start(bhid_t[:], bhid_d)
            nc.sync.dma_start(bout_t[:], bout_d)

            xb = []
            for i in range(3):
                xbt = xp.tile([128, 2 * BR * BC], F32R, tag=f"xb{i}")
                xb.append(xbt)
            for b in xb:
                nc.gpsimd.memset(b[:].bitcast(F32), 0.0)

            def xview(t):
                return t[:].rearrange("p (ch r c) -> p ch r c", ch=2, c=BC)

            for _rep in range(reps):
                # ---- layer 1: w_in (cin=64, K=64) -> x1 in xb[0] ----
                l = 1
                w_l = _wl(l)              # 52
                b0 = 2 + 2 * l
                x0v = x0_t[:].rearrange("p (r c) -> p r c", c=BC)
                win_t = wp.tile([128, 25 * 2 * 128], F32R, tag="whid")
                nc.sync.dma_start(win_t[:, :3840], win_d)
                # tap-pair packing: partitions 64-127 of x0 hold a 1-col-left-shifted
                # copy, so one K=128 matmul evaluates taps (ki,kj) and (ki,kj+1).
                for o in range(2):
                    for r0 in range(0, H, 8):
                        nr = 8
                        pt = ps.tile([128, nr * w_l], F32, tag=f"ps{(r0 // 8) % 2}")
                        mi = 0
                        for ki in range(K):
                            di = ki - 2
                            for si, dj in enumerate((-2, 0, 2)):
                                rhs = x0v[:, r0 + 2 + di: r0 + 2 + di + nr,
                                          b0 + dj: b0 + dj + w_l]
                                nc.tensor.matmul(
                                    pt[:],
                                    win_t[:, ((ki * 3 + si) * 2 + o) * 128:
                                          ((ki * 3 + si) * 2 + o + 1) * 128],
                                    rhs, start=(mi == 0), stop=(mi == 14))
                                mi += 1
                        dst = xview(xb[0])[:, o, r0 + 2: r0 + 2 + nr, b0: b0 + w_l]
                        tmp = evp.tile([128, nr * w_l], F32, tag="evt")
                        nc.scalar.activation(tmp[:], pt[:], AF.Relu, bias=bin_t[:, o:o + 1])
                        nc.vector.tensor_mul(
                            dst, tmp[:].rearrange("p (r c) -> p r c", c=w_l),
                            mskv[:, r0 + 2: r0 + 2 + nr, b0: b0 + w_l])

                # ---- layers 2..11: hid convs ----
                for l in range(2, 12):
                    hw_l = l - 2
                    w_l = _wl(l)
                    b0 = 2 + 2 * l
                    src = xb[(l - 2) % 3]
                    dst_b = xb[(l - 1) % 3]
                    res_b = xb[(l - 3) % 3] if (l % 2 == 1) else None
                    xv = xview(src)
                    if w_l >= 32:
                        chunks = [(r, 8) for r in range(0, H, 8)]
                    elif w_l >= 16:
                        chunks = [(0, 16), (16, 16)]
                    else:
                        chunks = [(0, 32)]
                    for o in range(2):
                        wt = wp.tile([128, 25 * 2 * 128], F32R, tag="whid")
                        nc.sync.dma_start(wt[:], whid_d[hw_l, o])
                        mms = [(ti, ci) for ti in range(25) for ci in range(2)
                               if not (o == 0 and ci == 1 and TAPS[ti][2] + TAPS[ti][3] >= 0)]
                        for kidx, (r0, nr) in enumerate(chunks):
                            pt = ps.tile([128, nr * w_l], F32, tag=f"ps{kidx % 2}")
                            for mi, (ti, ci) in enumerate(mms):
                                di, dj = TAPS[ti][2], TAPS[ti][3]
                                rhs = xv[:, ci, r0 + 2 + di: r0 + 2 + di + nr,
                                         b0 + dj: b0 + dj + w_l]
                                nc.tensor.matmul(
                                    pt[:], wt[:, (ti * 2 + ci) * 128:(ti * 2 + ci + 1) * 128],
                                    rhs, start=(mi == 0), stop=(mi == len(mms) - 1))
                            ptv = pt[:].rearrange("p (r c) -> p r c", c=w_l)
                            dstv = xview(dst_b)[:, o, r0 + 2: r0 + 2 + nr, b0: b0 + w_l]
                            bias_ap = bhid_t[:, hw_l * 2 + o: hw_l * 2 + o + 1]
                            mv = mskv[:, r0 + 2: r0 + 2 + nr, b0: b0 + w_l]
                            tmp = evp.tile([128, nr * w_l], F32, tag="evt")
                            nc.scalar.activation(tmp[:], pt[:], AF.Relu, bias=bias_ap)
                            tmpv = tmp[:].rearrange("p (r c) -> p r c", c=w_l)
                            if res_b is None:
                                nc.vector.tensor_mul(dstv, tmpv, mv)
                            else:
                                tmp2 = evp.tile([128, nr * w_l], F32, tag="evt2")
                                nc.vector.tensor_mul(tmp2[:], tmp[:],
                                                     mskv[:, r0 + 2: r0 + 2 + nr, b0: b0 + w_l])
                                resv = xview(res_b)[:, o, r0 + 2: r0 + 2 + nr, b0: b0 + w_l]
                                nc.vector.tensor_add(
                                    dstv, tmp2[:].rearrange("p (r c) -> p r c", c=w_l), resv)

                # ---- layer 12: w_out -> y = [mu | sig_raw], [128, 32*8] ----
                l = 12
                b0 = 2 + 2 * l            # 26
                xv = xview(xb[(l - 2) % 3])
                wout_t = wp.tile([128, 25 * 2 * 128], F32R, tag="whid")
                nc.sync.dma_start(wout_t[:], wout_d)
                y_t = gm.tile([128, 256], F32, tag="y")
                pt = ps.tile([128, 256], F32, tag="ps0")
                mi = 0
                for ti, (ki, kj, di, dj) in enumerate(TAPS):
                    for ci in range(2):
                        rhs = xv[:, ci, 2 + di: 2 + di + H, b0 + dj: b0 + dj + STRIP]
                        nc.tensor.matmul(
                            pt[:], wout_t[:, (ti * 2 + ci) * 128:(ti * 2 + ci + 1) * 128],
                            rhs, start=(mi == 0), stop=(mi == 49))
                        mi += 1
                nc.scalar.activation(y_t[:], pt[:], AF.Identity, bias=bout_t[:, :])

                # ---- GMM tail on [64, 256] tiles ----
                mu = y_t[0:64, :]
                s_t = gm.tile([64, 256], F32, tag="sraw")
                nc.sync.dma_start(s_t[:], y_t[64:128, :])   # realign sig to partitions 0..63
                ab = gm.tile([64, 256], F32, tag="ab")
                ex = gm.tile([64, 256], F32, tag="ex")
                ln = gm.tile([64, 256], F32, tag="ln")
                rl = gm.tile([64, 256], F32, tag="rl")
                sg = gm.tile([64, 256], F32, tag="sg")
                rc = gm.tile([64, 256], F32, tag="rc")
                nc.scalar.activation(ab[:], s_t[:], AF.Abs)
                nc.scalar.activation(ex[:], ab[:], AF.Exp, scale=-1.0)
                nc.scalar.activation(ln[:], ex[:], AF.Ln, bias=1.0)
                nc.scalar.activation(rl[:], s_t[:], AF.Relu)
                nc.vector.scalar_tensor_tensor(sg[:], rl[:], 1e-6, ln[:], AL.add, AL.add)
                nc.vector.reciprocal(rc[:], sg[:])
                big = gm.tile([64, 256 * BIN], F32, tag="big")
                inv_sqrt2 = float(1.0 / np.sqrt(2.0))
                for k in range(BIN):
                    e_k = float(k) + 0.5 - BIAS
                    tk = gm.tile([64, 256], F32, tag="tk")
                    zk = gm.tile([64, 256], F32, tag="zk")
                    ek = gm.tile([64, 256], F32, tag="ek")
                    nc.vector.tensor_scalar(tk[:], mu, -inv_sqrt2, e_k * inv_sqrt2,
                                            AL.mult, AL.add)
                    nc.vector.tensor_mul(zk[:], tk[:], rc[:])
                    nc.scalar.activation(ek[:], zk[:], AF.Erf)
                    dstk = big[:].rearrange("p (px k) -> p px k", k=BIN)[:, :, k]
                    nc.vector.tensor_scalar(dstk, ek[:], 32768.0, 32768.0, AL.mult, AL.add)
                nc.sync.dma_start(out_d, big[:])

    nc.compile()
    return nc


def _host_prep(data, mask, w_in, b_in, w_hid, b_hid, w_out, b_out):
    data, mask, w_in, b_in, w_hid, b_hid, w_out, b_out = [
        np.asarray(a, dtype=np.float32)
        for a in (data, mask, w_in, b_in, w_hid, b_hid, w_out, b_out)]
    m_in = _group_mask(CPN, 1, True)
    m_hid = _group_mask(CPN, CPN, False)
    m_out = _group_mask(NMIX, CPN, False)

    wm_in = (w_in * m_in).astype(np.float32)          # (256, 64, 5, 5)
    wm_hid = (w_hid * m_hid).astype(np.float32)       # (10, 256, 256, 5, 5)
    wm_out = (w_out * m_out).astype(np.float32)       # (192, 256, 5, 5)

    idx = np.concatenate([np.arange(G) * 3 + 1, np.arange(G) * 3 + 2])
    wo = wm_out[idx]                                  # (128, 256, 5, 5): [mu | sig]
    bo = b_out[idx].astype(np.float32)

    # pair-packed w_in: [p(128), (ki, s, o, col)]; s=0:(kj 0,1), s=1:(kj 2,3), s=2:(kj 4,-)
    wiT = wm_in.reshape(2, 128, 64, K, K).transpose(2, 3, 4, 0, 1)  # (p64, ki, kj, o, col)
    A_in = np.zeros((128, K, 3, 2, 128), np.float32)
    for si, kjs in enumerate(((0, 1), (2, 3), (4,))):
        A_in[0:64, :, si] = wiT[:, :, kjs[0]]
        if len(kjs) > 1:
            A_in[64:128, :, si] = wiT[:, :, kjs[1]]
    A_in = np.ascontiguousarray(A_in).reshape(128, K * 3 * 2 * 128)
    A_hid = np.ascontiguousarray(
        wm_hid.reshape(10, 2, 128, 2, 128, K, K).transpose(0, 1, 4, 5, 6, 3, 2)
    ).reshape(10, 2, 128, 25 * 2 * 128)
    A_out = np.ascontiguousarray(
        wo.reshape(128, 2, 128, K, K).transpose(2, 3, 4, 1, 0)).reshape(128, 25 * 2 * 128)

    bi = np.ascontiguousarray(b_in.astype(np.float32).reshape(2, 128).T)        # [128, 2]
    bh = np.ascontiguousarray(
        b_hid.astype(np.float32).reshape(10, 2, 128).transpose(2, 0, 1)).reshape(128, 20)

    tdata = ((data - BIAS) * mask)[0].astype(np.float32)   # (64, 32, 64)

    msks = []
    for c in range(NCORES):
        lo = STRIP * c - 24
        colm = np.zeros(BC, np.float32)
        for b in range(BC):
            col = lo + (b - 2)
            if 0 <= col < W:
                colm[b] = 1.0
        m = np.broadcast_to(colm, (128, BR, BC)).reshape(128, BR * BC)
        msks.append(np.ascontiguousarray(m))

    x0s = []
    for c in range(NCORES):
        buf = np.zeros((128, BR, BC), np.float32)
        lo = STRIP * c - 24
        s0, s1 = max(0, lo), min(W, lo + 56)
        if s1 > s0:
            buf[0:64, 2:2 + H, 2 + (s0 - lo): 2 + (s1 - lo)] = tdata[:, :, s0:s1]
        buf[64:128, :, :-1] = buf[0:64, :, 1:]   # shifted copy for tap-pair packing
        x0s.append(buf.reshape(128, BR * BC))
    return A_in, A_hid, A_out, bi, bh, bo.reshape(128, 1), x0s, msks


def _subprocess_fallback(data, mask, w_in, b_in, w_hid, b_hid, w_out, b_out, prec):
    """Run the kernel in a fresh process (fresh PJRT client) — recovers from a
    wedged accelerator state that in-process retries cannot clear."""
    import os
    import subprocess
    import tempfile
    import time as _time
    here = os.path.abspath(__file__)
    last = None
    for _try in range(3):
        with tempfile.TemporaryDirectory() as td:
            np.savez(os.path.join(td, "in.npz"), data=data, mask=mask, w_in=w_in,
                     b_in=b_in, w_hid=w_hid, b_hid=b_hid, w_out=w_out, b_out=b_out)
            r = subprocess.run(
                [sys.executable, here, "--subproc", os.path.join(td, "in.npz"),
                 os.path.join(td, "out.npy"), prec],
                capture_output=True, text=True)
            if r.returncode == 0 and os.path.exists(os.path.join(td, "out.npy")):
                return np.load(os.path.join(td, "out.npy"))
            last = r.stderr[-2000:] if r.stderr else "?"
            _time.sleep(5)
    raise RuntimeError(f"kernel subprocess fallback failed: {last}")


def kernel(data, mask, w_in, b_in, w_hid, b_hid, w_out, b_out, _reps=1, _prec="r"):
    from concourse.bass_utils import run_bass_kernel_spmd

    key = ("nc", _reps, _prec)
    if key not in _CACHE:
        _CACHE[key] = _build(_reps, _prec)
    nc = _CACHE[key]

    A_in, A_hid, A_out, bi, bh, bo, x0s, msks = _host_prep(
        data, mask, w_in, b_in, w_hid, b_hid, w_out, b_out)

    in_maps = [dict(x0=x0s[c], win=A_in, whid=A_hid, wout=A_out,
                    bin=bi, bhid=bh, bout=bo, msk=msks[c]) for c in range(NCORES)]
    import time as _time
    res = None
    for _try in range(3):
        try:
            res = run_bass_kernel_spmd(nc, in_maps, core_ids=list(range(NCORES)))
            break
        except Exception:
            # a wedged exec unit usually resets on the next (failed) load
            if _try == 2:
                return _subprocess_fallback(data, mask, w_in, b_in, w_hid, b_hid,
                                            w_out, b_out, _prec)
            _time.sleep(5)

    out = np.zeros((G, H, W, BIN), np.float32)
    for c in range(NCORES):
        out[:, :, STRIP * c:STRIP * (c + 1), :] = \
            res.results[c]["out"].reshape(G, H, STRIP, BIN)
    return out


if __name__ == "__main__" and len(sys.argv) >= 5 and sys.argv[1] == "--subproc":
    _i = dict(np.load(sys.argv[2]))
    _o = kernel(_i["data"], _i["mask"], _i["w_in"], _i["b_in"], _i["w_hid"],
                _i["b_hid"], _i["w_out"], _i["b_out"], _prec=sys.argv[4])
    np.save(sys.argv[3], _o)
    sys.exit(0)

if __name__ == "__main__":
    inp = dict(np.load("/root/problem/ref_inp.npz"))
    ref = np.load("/root/problem/ref_out.npy")
    out = kernel(**inp)
    d = out - ref
    print("rel (norm):", np.linalg.norm(d) / np.linalg.norm(ref))
    print("absmax:", np.abs(d).max())
    bad = np.abs(d) > 64
    print("frac |err|>64:", bad.mean())
